# revision 2
# baseline (speedup 1.0000x reference)
"""CRY gate kernel for Trainium2 (Bass/Tile), 8-core SPMD.

The reference builds a sparse 4096x4096 complex unitary U for a controlled-RY
gate (control = wire 0 = MSB, target = wire 1) and computes U @ x.  The gate
structure collapses to:

    rows [0, 2048)          : identity
    rows A=[2048, 3072) and B=[3072, 4096), paired r <-> r+1024:
        yA =  c*A - s*B
        yB = -s*A + c*B        with c = cos(theta/2), s = sin(theta/2)

applied independently to the real and imaginary parts (U is real).

Sharding: data-parallel over the batch B=128 -> 16 columns per core; theta is
replicated and sin/cos are computed on-device on the Vector engine (magic-number
round + odd minimax polynomial for sin(2*pi*f); avoids ACT table loads).
"""

import sys

import numpy as np

for _p in ("/opt/trn_rl_repo",):
    if _p not in sys.path:
        sys.path.insert(0, _p)

D = 4096
BATCH = 128
NCORES = 8
BL = BATCH // NCORES  # 16 columns per core
P = 128
H = 2048  # identity rows
Q = 1024  # rotation block size
FA = Q * BL // P  # free-dim per component block = 128

# sin(2*pi*f) = f * sum_k KSIN[k] * (f^2)^k  for f in [-0.5, 0.5]
KSIN = [
    6.2831827932940385,
    -41.341419375071474,
    81.59613848541618,
    -76.57968507422851,
    41.20373129237858,
    -12.268840194963111,
]
MAGIC = 12582912.0  # 1.5 * 2^23: (x + MAGIC) - MAGIC == round(x) in fp32
INV_4PI = float(1.0 / (4.0 * np.pi))

_STATE: dict = {}


def _build_nc():
    import concourse.bacc as bacc
    import concourse.mybir as mybir
    from concourse.tile import TileContext

    f32 = mybir.dt.float32
    mult = mybir.AluOpType.mult
    add = mybir.AluOpType.add
    sub = mybir.AluOpType.subtract

    nc = bacc.Bacc("TRN2", target_bir_lowering=False, debug=False)
    xr = nc.dram_tensor("xr", [D, BL], f32, kind="ExternalInput").ap()
    xi = nc.dram_tensor("xi", [D, BL], f32, kind="ExternalInput").ap()
    th = nc.dram_tensor("th", [1], f32, kind="ExternalInput").ap()
    yr = nc.dram_tensor("yr", [D, BL], f32, kind="ExternalOutput").ap()
    yi = nc.dram_tensor("yi", [D, BL], f32, kind="ExternalOutput").ap()

    def blk(t, r0):
        # rows [r0, r0+Q) of a (D, BL) DRAM tensor as a (128, FA) AP;
        # partition p covers rows r0+8p .. r0+8p+7 (512B contiguous each)
        return t[r0 : r0 + Q, :].rearrange("(p r) c -> p (r c)", p=P)

    with TileContext(nc) as tc:
        with tc.tile_pool(name="pool", bufs=1) as pool:
            # --- identity rows: straight DRAM->DRAM copies ---
            nc.sync.dma_start(out=yr[0:H, :], in_=xr[0:H, :])
            nc.sync.dma_start(out=yi[0:H, :], in_=xi[0:H, :])

            # --- c = cos(theta/2), s = sin(theta/2) as [128,1] scalars ---
            thb = pool.tile([P, 1], f32)
            nc.sync.dma_start(out=thb[:], in_=th.to_broadcast((P, 1)))

            v = pool.tile([P, 1], f32)
            nc.vector.tensor_scalar(v[:], thb[:], INV_4PI, None, mult)
            t1 = pool.tile([P, 1], f32)
            nc.vector.tensor_scalar(t1[:], v[:], MAGIC, None, add)
            r1 = pool.tile([P, 1], f32)
            nc.vector.tensor_scalar(r1[:], t1[:], MAGIC, None, sub)
            f = pool.tile([P, 1], f32)
            nc.vector.tensor_sub(f[:], v[:], r1[:])

            fcr = pool.tile([P, 1], f32)
            nc.vector.tensor_scalar(fcr[:], f[:], 0.25, None, add)
            t2 = pool.tile([P, 1], f32)
            nc.vector.tensor_scalar(t2[:], fcr[:], MAGIC, None, add)
            r2 = pool.tile([P, 1], f32)
            nc.vector.tensor_scalar(r2[:], t2[:], MAGIC, None, sub)
            fc = pool.tile([P, 1], f32)
            nc.vector.tensor_sub(fc[:], fcr[:], r2[:])

            def sin2pi(fv, label):
                z = pool.tile([P, 1], f32, tag=f"z_{label}")
                nc.vector.tensor_mul(z[:], fv[:], fv[:])
                p0 = pool.tile([P, 1], f32, tag=f"p_{label}")
                nc.vector.tensor_scalar(p0[:], z[:], KSIN[5], KSIN[4], mult, add)
                for i, kk in enumerate((KSIN[3], KSIN[2], KSIN[1], KSIN[0])):
                    pn = pool.tile([P, 1], f32, tag=f"p_{label}_{i}")
                    nc.vector.tensor_scalar(pn[:], p0[:], z[:], kk, mult, add)
                    p0 = pn
                out = pool.tile([P, 1], f32, tag=f"s_{label}")
                nc.vector.tensor_mul(out[:], p0[:], fv[:])
                return out

            s = sin2pi(f, "sin")  # sin(theta/2)
            c = sin2pi(fc, "cos")  # cos(theta/2)

            # --- rotation blocks: real+imag packed in free dim [128, 256] ---
            A = pool.tile([P, 2 * FA], f32)
            Bt = pool.tile([P, 2 * FA], f32)
            nc.sync.dma_start(out=A[:, 0:FA], in_=blk(xr, H))
            nc.sync.dma_start(out=A[:, FA : 2 * FA], in_=blk(xi, H))
            nc.sync.dma_start(out=Bt[:, 0:FA], in_=blk(xr, H + Q))
            nc.sync.dma_start(out=Bt[:, FA : 2 * FA], in_=blk(xi, H + Q))

            cA = pool.tile([P, 2 * FA], f32)
            sB = pool.tile([P, 2 * FA], f32)
            yA = pool.tile([P, 2 * FA], f32)
            nc.vector.tensor_scalar(cA[:], A[:], c[:], None, mult)
            nc.vector.tensor_scalar(sB[:], Bt[:], s[:], None, mult)
            nc.vector.tensor_sub(yA[:], cA[:], sB[:])
            nc.sync.dma_start(out=blk(yr, H), in_=yA[:, 0:FA])
            nc.sync.dma_start(out=blk(yi, H), in_=yA[:, FA : 2 * FA])

            sA = pool.tile([P, 2 * FA], f32)
            cB = pool.tile([P, 2 * FA], f32)
            yB = pool.tile([P, 2 * FA], f32)
            nc.vector.tensor_scalar(sA[:], A[:], s[:], None, mult)
            nc.vector.tensor_scalar(cB[:], Bt[:], c[:], None, mult)
            nc.vector.tensor_sub(yB[:], cB[:], sA[:])
            nc.sync.dma_start(out=blk(yr, H + Q), in_=yB[:, 0:FA])
            nc.sync.dma_start(out=blk(yi, H + Q), in_=yB[:, FA : 2 * FA])

    nc.compile()
    return nc


def _get_nc():
    if "nc" not in _STATE:
        _STATE["nc"] = _build_nc()
    return _STATE["nc"]


def _run(xr, xi, th, **kwargs):
    """Run the SPMD kernel on 8 cores. Returns (y_complex, BassKernelResults)."""
    from concourse.bass_utils import run_bass_kernel_spmd

    nc = _get_nc()
    in_maps = [
        {
            "xr": np.ascontiguousarray(xr[:, k * BL : (k + 1) * BL]),
            "xi": np.ascontiguousarray(xi[:, k * BL : (k + 1) * BL]),
            "th": th,
        }
        for k in range(NCORES)
    ]
    out = run_bass_kernel_spmd(nc, in_maps, list(range(NCORES)), **kwargs)
    yr = np.concatenate([out.results[k]["yr"] for k in range(NCORES)], axis=1)
    yi = np.concatenate([out.results[k]["yi"] for k in range(NCORES)], axis=1)
    y = yr.astype(np.complex64)
    y.imag = yi
    return y, out


def kernel(x_real, x_imag, theta):
    xr = np.ascontiguousarray(np.asarray(x_real, dtype=np.float32))
    xi = np.ascontiguousarray(np.asarray(x_imag, dtype=np.float32))
    th = np.ascontiguousarray(np.asarray(theta, dtype=np.float32)).reshape(1)
    y, _ = _run(xr, xi, th)
    return y


# revision 11
# speedup vs baseline: 1.1459x; 1.1459x over previous
"""CRY gate kernel for Trainium2 (raw Bass/Bacc), 8-core SPMD.

The reference builds a sparse 4096x4096 complex unitary U for a controlled-RY
gate (control = wire 0 = MSB, target = wire 1) and computes U @ x.  The gate
structure collapses to:

    rows [0, 2048)          : identity
    rows A=[2048, 3072) and B=[3072, 4096), paired r <-> r+1024:
        yA =  c*A - s*B
        yB = -s*A + c*B        with c = cos(theta/2), s = sin(theta/2)

applied independently to the real and imaginary parts (U is real).

Sharding: data-parallel over the batch 128 -> 16 columns per core; theta is
replicated and sin/cos are computed on-device on the Vector engine
(magic-number round + odd minimax polynomial for sin(2*pi*f), both lanes
packed in a [128,2] tile).

Raw Bacc (no TileContext) to avoid the multi-microsecond kernel-tail
drain/barrier butterfly.  DMAs are spread over the Sync / Scalar / GpSimd
sequencers so their issue costs overlap:

    gpsimd: {0, 0.25} lane consts; yr/yi[0:2048] <- xr/xi[0:2048] DRAM->DRAM;
            final semaphore clear (re-execution safety)
    sync  : theta -> SBUF bcast; xr[2048:4096] -> Xr; Xr -> yr[2048:4096]
    scalar: xi[2048:4096] -> Xi; Xi -> yi[2048:4096]
    vector: sin/cos chain, then per component:
              P = s * [B|A]   (two half-width tensor_scalar ops)
              X <- (X * c) - P   (one fused scalar_tensor_tensor, in place)

Same-engine RAW hazards on the pipelined DVE are ordered with a single chain
semaphore (then_inc on the producer, fused wait on the consumer); the store
DMAs wait on the chain semaphore values of the final in-place rotations.

Load layout: rows 2048:4096 rearranged "(h p r) c -> p h (r c)" so partition p
holds A-rows 2048+8p..+7 in cols 0:128 and B-rows 3072+8p..+7 in cols 128:256
(pairs lane-aligned, 512B contiguous chunks per partition).
"""

import sys

import numpy as np

for _p in ("/opt/trn_rl_repo",):
    if _p not in sys.path:
        sys.path.insert(0, _p)

D = 4096
BATCH = 128
NCORES = 8
BL = BATCH // NCORES  # 16 columns per core
P = 128
H = 2048  # identity rows
Q = 1024  # rotation block size
FA = Q * BL // P  # free-dim per component block = 128

# sin(2*pi*f) = f * sum_k KSIN[k] * (f^2)^k  for f in [-0.5, 0.5]  (deg 4,
# max abs err ~6e-6)
KSIN = [
    6.283054082191078,
    -41.331122580391586,
    81.36549238026443,
    -74.47093984475363,
    32.76882701641142,
]
MAGIC = 12582912.0  # 1.5 * 2^23: (x + MAGIC) - MAGIC == round(x) in fp32
INV_4PI = float(1.0 / (4.0 * np.pi))

_STATE: dict = {}


def _build_nc():
    import concourse.bacc as bacc
    import concourse.mybir as mybir

    f32 = mybir.dt.float32
    mult = mybir.AluOpType.mult
    add = mybir.AluOpType.add
    sub = mybir.AluOpType.subtract

    nc = bacc.Bacc("TRN2", target_bir_lowering=False, debug=False)
    xr = nc.dram_tensor("xr", [D, BL], f32, kind="ExternalInput").ap()
    xi = nc.dram_tensor("xi", [D, BL], f32, kind="ExternalInput").ap()
    th = nc.dram_tensor("th", [1], f32, kind="ExternalInput").ap()
    yr = nc.dram_tensor("yr", [D, BL], f32, kind="ExternalOutput").ap()
    yi = nc.dram_tensor("yi", [D, BL], f32, kind="ExternalOutput").ap()

    def pairs(t):
        # rows [H, D) as [128, 2, 128]: [:, 0, :] = A rows, [:, 1, :] = B rows,
        # pair index = partition
        return t[H:D, :].rearrange("(h p r) c -> p h (r c)", h=2, p=P)

    def halves(t):
        # matching [128, 2, 128] view of a [128, 256] SBUF tile
        return t.rearrange("p (h f) -> p h f", h=2)

    # SBUF tiles (persistent allocations)
    thb = nc.alloc_sbuf_tensor("thb", [P, 1], f32).ap()
    c01 = nc.alloc_sbuf_tensor("c01", [P, 2], f32).ap()
    v2 = nc.alloc_sbuf_tensor("v2", [P, 2], f32).ap()
    t1 = nc.alloc_sbuf_tensor("t1", [P, 2], f32).ap()
    r1 = nc.alloc_sbuf_tensor("r1", [P, 2], f32).ap()
    g = nc.alloc_sbuf_tensor("g", [P, 2], f32).ap()
    z = nc.alloc_sbuf_tensor("z", [P, 2], f32).ap()
    p0 = nc.alloc_sbuf_tensor("p0", [P, 2], f32).ap()
    p1 = nc.alloc_sbuf_tensor("p1", [P, 2], f32).ap()
    sc = nc.alloc_sbuf_tensor("sc", [P, 2], f32).ap()
    Xr = nc.alloc_sbuf_tensor("Xr", [P, 2 * FA], f32).ap()
    Xi = nc.alloc_sbuf_tensor("Xi", [P, 2 * FA], f32).ap()
    Pr = nc.alloc_sbuf_tensor("Pr", [P, 2 * FA], f32).ap()
    Pi = nc.alloc_sbuf_tensor("Pi", [P, 2 * FA], f32).ap()

    # semaphores (contiguous range right after bass's built-ins)
    sems = [nc.alloc_semaphore(n) for n in (
        "th_sem", "ldr_sem", "ldi_sem", "g_sem",
        "str_sem", "sti_sem", "d2d_sem", "csem",
    )]
    th_sem, ldr_sem, ldi_sem, g_sem, str_sem, sti_sem, d2d_sem, csem = sems
    sem_lo = min(s.num for s in sems)
    sem_hi = max(s.num for s in sems)
    assert sem_hi - sem_lo + 1 == len(sems), [s.num for s in sems]

    # --- GpSimd: lane consts, then identity rows (DRAM->DRAM, no deps) ---
    nc.gpsimd.memset(c01[:, 0:1], 0.0).then_inc(g_sem, 1)
    nc.gpsimd.memset(c01[:, 1:2], 0.25).then_inc(g_sem, 1)
    nc.gpsimd.dma_start(out=yr[0:H, :], in_=xr[0:H, :]).then_inc(d2d_sem, 16)
    nc.gpsimd.dma_start(out=yi[0:H, :], in_=xi[0:H, :]).then_inc(d2d_sem, 16)

    # --- Sync sequencer: xr load, yr store ---
    nc.sync.dma_start(out=halves(Xr), in_=pairs(xr)).then_inc(ldr_sem, 16)
    nc.sync.wait_ge(csem, 16)  # Xr rotation done (implies load consumed)
    nc.sync.dma_start(out=pairs(yr), in_=halves(Xr)).then_inc(str_sem, 16)

    # --- Scalar sequencer: theta bcast (tiny, first), xi load, yi store ---
    nc.scalar.dma_start(out=thb, in_=th.to_broadcast((P, 1))).then_inc(th_sem, 16)
    nc.scalar.dma_start(out=halves(Xi), in_=pairs(xi)).then_inc(ldi_sem, 16)
    nc.scalar.wait_ge(csem, 19)  # Xi rotation done
    nc.scalar.dma_start(out=pairs(yi), in_=halves(Xi)).then_inc(sti_sem, 16)

    # --- Vector engine: sin/cos chain + rotations, chained via csem ---
    V = nc.vector
    n = 0

    def step(emit, *waits):
        nonlocal n
        for sem, val in waits:
            V.wait_ge(sem, val)
        if n:
            V.wait_ge(csem, n)
        inst = emit()
        inst.then_inc(csem, 1)
        n += 1
        return inst

    # lanes {v, v+0.25} with v = theta/(4*pi)
    step(lambda: V.scalar_tensor_tensor(
        v2, thb.to_broadcast((P, 2)), INV_4PI, c01, mult, add),
        (th_sem, 16), (g_sem, 2))
    step(lambda: V.tensor_scalar(t1, v2, MAGIC, None, add))
    step(lambda: V.tensor_scalar(r1, t1, MAGIC, None, sub))  # round(v2)
    step(lambda: V.tensor_sub(g, v2, r1))  # wrapped to [-0.5, 0.5]
    step(lambda: V.tensor_mul(z, g, g))
    step(lambda: V.tensor_scalar(p0, z, KSIN[4], KSIN[3], mult, add))
    a, b = p0, p1
    for kk in (KSIN[2], KSIN[1], KSIN[0]):
        step(lambda a=a, b=b: V.tensor_mul(b, a, z))
        step(lambda a=a, b=b, kk=kk: V.tensor_scalar(a, b, kk, None, add))
    step(lambda: V.tensor_mul(sc, p0, g))  # lanes {sin(th/2), cos(th/2)}
    assert n == 13, n
    s_ap = sc[:, 0:1]
    c_ap = sc[:, 1:2]

    step(lambda: V.tensor_scalar(Pr[:, 0:FA], Xr[:, FA : 2 * FA], s_ap, None, mult),
         (ldr_sem, 16))
    step(lambda: V.tensor_scalar(Pr[:, FA : 2 * FA], Xr[:, 0:FA], s_ap, None, mult))
    step(lambda: V.scalar_tensor_tensor(Xr, Xr, c_ap, Pr, mult, sub))
    assert n == 16, n
    step(lambda: V.tensor_scalar(Pi[:, 0:FA], Xi[:, FA : 2 * FA], s_ap, None, mult),
         (ldi_sem, 16))
    step(lambda: V.tensor_scalar(Pi[:, FA : 2 * FA], Xi[:, 0:FA], s_ap, None, mult))
    step(lambda: V.scalar_tensor_tensor(Xi, Xi, c_ap, Pi, mult, sub))
    assert n == 19, n

    # --- GpSimd tail: wait for every completion, clear our semaphores ---
    # (most of these are transitively implied by str/sti, but the race
    # detector wants a direct wait on every cleared semaphore's updates)
    nc.gpsimd.wait_ge(th_sem, 16)
    nc.gpsimd.wait_ge(ldr_sem, 16)
    nc.gpsimd.wait_ge(ldi_sem, 16)
    nc.gpsimd.wait_ge(g_sem, 2)
    nc.gpsimd.wait_ge(csem, 19)
    nc.gpsimd.wait_ge(str_sem, 16)
    nc.gpsimd.wait_ge(sti_sem, 16)
    nc.gpsimd.wait_ge(d2d_sem, 32)
    # one light barrier so the clear is globally ordered (the dedicated
    # barrier sems return to 0 by design, so they need no clearing)
    nc.all_engine_barrier()
    nc.gpsimd.sem_clear(range(sem_lo, sem_hi + 1))

    nc.compile()
    return nc


def _get_nc():
    if "nc" not in _STATE:
        _STATE["nc"] = _build_nc()
    return _STATE["nc"]


def _run(xr, xi, th, **kwargs):
    """Run the SPMD kernel on 8 cores. Returns (y_complex, BassKernelResults)."""
    from concourse.bass_utils import run_bass_kernel_spmd

    nc = _get_nc()
    in_maps = [
        {
            "xr": np.ascontiguousarray(xr[:, k * BL : (k + 1) * BL]),
            "xi": np.ascontiguousarray(xi[:, k * BL : (k + 1) * BL]),
            "th": th,
        }
        for k in range(NCORES)
    ]
    out = run_bass_kernel_spmd(nc, in_maps, list(range(NCORES)), **kwargs)
    yr = np.concatenate([out.results[k]["yr"] for k in range(NCORES)], axis=1)
    yi = np.concatenate([out.results[k]["yi"] for k in range(NCORES)], axis=1)
    y = yr.astype(np.complex64)
    y.imag = yi
    return y, out


def kernel(x_real, x_imag, theta):
    xr = np.ascontiguousarray(np.asarray(x_real, dtype=np.float32))
    xi = np.ascontiguousarray(np.asarray(x_imag, dtype=np.float32))
    th = np.ascontiguousarray(np.asarray(theta, dtype=np.float32)).reshape(1)
    y, _ = _run(xr, xi, th)
    return y
